# revision 1
# baseline (speedup 1.0000x reference)
"""BitLinear FFN (BitNet b1.58) Trainium2 kernel, 8-core SPMD.

Strategy: data-parallel over tokens. Every core processes 1024 of the 8192
tokens end-to-end. Weight quantization (+ transpose to contraction-major
layout, + cast to fp8e4) is sharded 1/8 per core and shared via three
pipelined AllGathers (one per matrix). All weights stay cached in SBUF.

Exactness: activation quantization produces integers in [-127, 127] (exact
in bf16) and weight quantization produces {-1, 0, 1} (exact in fp8e4); the
PE accumulates in fp32, so all three matmuls are exact integer arithmetic
(fp8 lhsT x bf16 rhs verified exact on hardware).
Per-token/per-tensor dequant scales are applied on the fly:
  gate = gate_int * c_g          (c_g per token, before silu)
  c_u cancels inside the second act-quant, so `up` stays in integer form
  out  = down_int * F_t          (F_t per token, fused into PSUM evacuation)
"""

import numpy as np

import concourse.bacc as bacc
import concourse.bass as bass
import concourse.bass_isa as bass_isa
import concourse.mybir as mybir
import concourse.tile as tile
from concourse.masks import make_identity

P = 128
HID = 1024
INNER = 4096
N_CORES = 8
T_CORE = 1024          # tokens per core
TC = 256               # token chunk in the main loop
NCH = T_CORE // TC     # 4 chunks
MT = TC // P           # 2 token tiles per chunk
KI = HID // P          # 8 contraction tiles for gate/up
KOG = INNER // P       # 32 contraction tiles for down
OSH = INNER // N_CORES  # 512, o-shard per core
HSH = HID // P         # 8 h-subtiles in w_down shard [1024, 512]

MROUND = 12582912.0    # 1.5 * 2**23: (v + M) - M == round-half-even(v)
W_ELEMS = float(INNER * HID)

F32 = mybir.dt.float32
BF16 = mybir.dt.bfloat16
FP8 = mybir.dt.float8e4

# element offsets inside the per-core staging buffer (bf16 elements)
WG_OFF = 0
WU_OFF = OSH * HID          # 524288
WD_OFF = 2 * OSH * HID      # 1048576
STAGE_ELEMS = 3 * OSH * HID  # 1572864

A = mybir.AluOpType
AF = mybir.ActivationFunctionType


def build_bass(sim_mode: bool = False, main_chunks: int = NCH, reps: int = 1):
    """Build the SPMD program. sim_mode replaces collectives with local
    stand-ins so the single-core cost-model simulator can run it."""
    nc = bacc.Bacc(
        "TRN2", target_bir_lowering=False, debug=False,
        num_devices=N_CORES,
    )
    groups = [list(range(N_CORES))]

    x_d = nc.dram_tensor("x_shard", [T_CORE, HID], F32, kind="ExternalInput")
    wg_d = nc.dram_tensor("wg_shard", [OSH, HID], F32, kind="ExternalInput")
    wu_d = nc.dram_tensor("wu_shard", [OSH, HID], F32, kind="ExternalInput")
    wd_d = nc.dram_tensor("wd_shard", [HID, OSH], F32, kind="ExternalInput")
    out_d = nc.dram_tensor("out_shard", [T_CORE, HID], F32, kind="ExternalOutput")

    wg_r = wg_d.ap().rearrange("(po p) i -> po p i", p=P)    # [4, 128, 1024]
    wu_r = wu_d.ap().rearrange("(po p) i -> po p i", p=P)
    wd_r = wd_d.ap().rearrange("(hs p) o -> hs p o", p=P)    # [8, 128, 512]
    x_r = x_d.ap().rearrange("(n p) i -> n p i", p=P)        # [8, 128, 1024]
    out_r = out_d.ap().rearrange("(n p) h -> n p h", p=P)

    with tile.TileContext(nc) as tc:
        with (
            tc.tile_pool(name="const", bufs=1) as constp,
            tc.tile_pool(name="big", bufs=1) as bigp,
            tc.tile_pool(name="stream", bufs=2) as streamp,
            tc.tile_pool(name="stg", bufs=1) as stgp,
            tc.tile_pool(name="ew", bufs=2) as ewp,
            tc.tile_pool(name="outp", bufs=2) as outpp,
            tc.tile_pool(name="tiny", bufs=2) as tinyp,
            tc.tile_pool(name="pg", bufs=4, space="PSUM") as pgp,
            tc.tile_pool(name="pd", bufs=2, space="PSUM") as pdp,
            tc.tile_pool(name="pt", bufs=2, space="PSUM") as ptp,
            tc.tile_pool(name="dram", bufs=1, space="DRAM") as dramp,
        ):
            ident = constp.tile([P, P], BF16)
            make_identity(nc, ident)
            ones_col = constp.tile([P, 1], F32)
            nc.gpsimd.memset(ones_col[:], 1.0)

            def emit_body():
                # ---------------- |w| partial sums over this core's shards -----
                sums_col = constp.tile([P, 4], F32)
                nc.gpsimd.memset(sums_col[:], 0.0)
                for j, (src, n_sub) in enumerate(
                    ((wg_r, 4), (wu_r, 4), (wd_r, HSH))
                ):
                    for po in range(n_sub):
                        wld = streamp.tile([P, HID], F32, tag="wld")
                        nc.sync.dma_start(out=wld[:, :src.shape[2]], in_=src[po])
                        part = tinyp.tile([P, 1], F32, tag="wabs")
                        nc.vector.tensor_reduce(
                            out=part[:], in_=wld[:, :src.shape[2]],
                            axis=mybir.AxisListType.X,
                            op=A.add, apply_absolute_value=True)
                        nc.vector.tensor_tensor(
                            out=sums_col[:, j:j + 1], in0=sums_col[:, j:j + 1],
                            in1=part[:], op=A.add)

                psums = ptp.tile([1, 4], F32, tag="pt")
                nc.tensor.matmul(psums[:], lhsT=ones_col[:], rhs=sums_col[:],
                                 start=True, stop=True)
                sums_sb = tinyp.tile([1, 4], F32)
                nc.vector.tensor_copy(out=sums_sb[:], in_=psums[:])

                # tiny AllReduce of the three |w| sums
                sums_in = dramp.tile([1, 4], F32)
                sums_out = dramp.tile([1, 4], F32, addr_space="Shared")
                nc.sync.dma_start(out=sums_in[:], in_=sums_sb[:])
                if sim_mode:
                    nc.sync.dma_start(out=sums_out[:], in_=sums_in[:])
                else:
                    nc.gpsimd.collective_compute(
                        "AllReduce", A.add, replica_groups=groups,
                        ins=[sums_in[:]], outs=[sums_out[:]])
                sums_all = tinyp.tile([1, 4], F32)
                nc.sync.dma_start(out=sums_all[:], in_=sums_out[:])

                # clip-means (= 1/s_w) and s_w, broadcast to all partitions
                mcl = tinyp.tile([1, 4], F32)
                nc.vector.tensor_scalar(
                    out=mcl[:], in0=sums_all[:], scalar1=1.0 / W_ELEMS,
                    scalar2=1e-5, op0=A.mult, op1=A.max)
                sw = tinyp.tile([1, 4], F32)
                nc.vector.reciprocal(out=sw[:], in_=mcl[:])
                swb = constp.tile([P, 4], F32)
                nc.gpsimd.partition_broadcast(swb[:], sw[0:1, :])
                mclb = constp.tile([P, 4], F32)
                nc.gpsimd.partition_broadcast(mclb[:], mcl[0:1, :])
                # bc_ud = clip_mean_wu * clip_mean_wd / 127^2   (for F_t)
                bc_ud = constp.tile([P, 1], F32)
                nc.vector.tensor_tensor(
                    out=bc_ud[:], in0=mclb[:, 1:2], in1=mclb[:, 2:3], op=A.mult)
                nc.vector.tensor_scalar_mul(bc_ud[:], bc_ud[:], 1.0 / (127.0 * 127.0))

                # ---------------- quantize + transpose weight shards -----------
                shared_as = "Local" if sim_mode else "Shared"
                stg_wg = dramp.tile([OSH * HID], FP8)
                stg_wu = dramp.tile([OSH * HID], FP8)
                stg_wd = dramp.tile([OSH * HID], FP8)
                agt_wg = dramp.tile([N_CORES, OSH * HID], FP8, addr_space=shared_as)
                agt_wu = dramp.tile([N_CORES, OSH * HID], FP8, addr_space=shared_as)
                ag_wd = dramp.tile([N_CORES, OSH * HID], FP8, addr_space=shared_as)

                def do_ag(stg, ag):
                    if sim_mode:
                        for c in range(N_CORES):
                            nc.sync.dma_start(out=ag[c, :], in_=stg[:])
                    else:
                        nc.gpsimd.collective_compute(
                            "AllGather", A.bypass, replica_groups=groups,
                            ins=[stg[:]], outs=[ag[:]])

                def quant_tile(src_sb, w_idx, width):
                    """bf16 tile <- clip(round(src * s_w), -1, 1); src clobbered."""
                    sw_col = swb[:, w_idx:w_idx + 1]
                    nc.vector.tensor_scalar(
                        out=src_sb, in0=src_sb, scalar1=sw_col, scalar2=MROUND,
                        op0=A.mult, op1=A.add)
                    wq_b = streamp.tile([P, HID], BF16, tag="wqb")
                    nc.vector.tensor_scalar(
                        out=wq_b[:, :width], in0=src_sb, scalar1=-MROUND,
                        scalar2=1.0, op0=A.add, op1=A.min)
                    nc.vector.tensor_scalar_max(
                        wq_b[:, :width], wq_b[:, :width], -1.0)
                    return wq_b

                # gate / up: stage layout [KI, 128 i, OSH o]
                for src, stg, ag, w_idx in (
                    (wg_r, stg_wg, agt_wg, 0), (wu_r, stg_wu, agt_wu, 1),
                ):
                    stage_sb = stgp.tile([P, KI, OSH], FP8, tag="stg")
                    for po in range(4):
                        wld = streamp.tile([P, HID], F32, tag="wld")
                        nc.sync.dma_start(out=wld[:], in_=src[po])
                        wq_b = quant_tile(wld[:], w_idx, HID)
                        for ki in range(KI):
                            pt_t = ptp.tile([P, P], BF16, tag="pt")
                            nc.tensor.transpose(
                                pt_t[:], wq_b[:, ki * P:(ki + 1) * P], ident[:])
                            nc.vector.tensor_copy(
                                out=stage_sb[:, ki, po * P:(po + 1) * P], in_=pt_t[:])
                    nc.sync.dma_start(
                        out=stg[:].rearrange(
                            "(ki i o) -> i ki o", ki=KI, i=P, o=OSH),
                        in_=stage_sb[:])
                    do_ag(stg, ag)

                # down: stage layout [4 ko, 128 o, HID h]
                stage_sb = stgp.tile([P, 4, HID], FP8, tag="stg")
                for hs in range(HSH):
                    wld = streamp.tile([P, HID], F32, tag="wld")
                    nc.sync.dma_start(out=wld[:, :OSH], in_=wd_r[hs])
                    wq_b = quant_tile(wld[:, :OSH], 2, OSH)
                    for ko in range(4):
                        pt_t = ptp.tile([P, P], BF16, tag="pt")
                        nc.tensor.transpose(
                            pt_t[:], wq_b[:, ko * P:(ko + 1) * P], ident[:])
                        nc.vector.tensor_copy(
                            out=stage_sb[:, ko, hs * P:(hs + 1) * P], in_=pt_t[:])
                nc.sync.dma_start(
                    out=stg_wd[:].rearrange(
                        "(ko o h) -> o ko h", ko=4, o=P, h=HID),
                    in_=stage_sb[:])
                do_ag(stg_wd, ag_wd)

                ag_wg = [
                    agt_wg[c, :].rearrange(
                        "(ki i o) -> i ki o", ki=KI, i=P, o=OSH)
                    for c in range(N_CORES)
                ]
                ag_wu = [
                    agt_wu[c, :].rearrange(
                        "(ki i o) -> i ki o", ki=KI, i=P, o=OSH)
                    for c in range(N_CORES)
                ]

                # ---------------- x shard: quantize + transpose -----------------
                xqT = bigp.tile([P, KI, T_CORE], BF16, tag="xqT")   # [i, t]
                absm_c = constp.tile([P, KI], F32)                  # clip(absmax_x)

                for ts in range(KI):  # 8 token tiles of 128
                    x_sb = streamp.tile([P, HID], F32, tag="xld")
                    nc.sync.dma_start(out=x_sb[:], in_=x_r[ts])
                    am = tinyp.tile([P, 1], F32, tag="am")
                    nc.vector.tensor_reduce(
                        out=am[:], in_=x_sb[:], axis=mybir.AxisListType.X,
                        op=A.max, apply_absolute_value=True)
                    nc.vector.tensor_scalar_max(absm_c[:, ts:ts + 1], am[:], 1e-5)
                    s1c = tinyp.tile([P, 1], F32, tag="s1c")
                    nc.vector.reciprocal(out=s1c[:], in_=absm_c[:, ts:ts + 1])
                    nc.vector.tensor_scalar_mul(s1c[:], s1c[:], 127.0)
                    # xq = round(x * s1) in-place then cast
                    nc.vector.tensor_scalar(
                        out=x_sb[:], in0=x_sb[:], scalar1=s1c[:, 0:1],
                        scalar2=MROUND, op0=A.mult, op1=A.add)
                    xq_b = streamp.tile([P, HID], BF16, tag="wqb")
                    nc.vector.tensor_scalar(
                        out=xq_b[:], in0=x_sb[:], scalar1=-MROUND, scalar2=None,
                        op0=A.add)
                    for ki in range(KI):
                        pt_t = ptp.tile([P, P], BF16, tag="pt")
                        nc.tensor.transpose(
                            pt_t[:], xq_b[:, ki * P:(ki + 1) * P], ident[:])
                        nc.vector.tensor_copy(
                            out=xqT[:, ki, ts * P:(ts + 1) * P], in_=pt_t[:])

                # c_g: clip(absmax_x) * clip_mean_wg / 127, as [1, T] row
                cg_col = constp.tile([P, KI], F32)
                nc.vector.tensor_scalar(
                    out=cg_col[:], in0=absm_c[:], scalar1=mclb[:, 0:1],
                    scalar2=1.0 / 127.0, op0=A.mult, op1=A.mult)
                cg_row = constp.tile([1, T_CORE], F32)
                for ts in range(KI):
                    nc.sync.dma_start(
                        out=cg_row[0:1, ts * P:(ts + 1) * P],
                        in_=cg_col[:, ts:ts + 1])

                # ---------------- cached transposed w_down ---------------------
                wdt4 = bigp.tile([P, N_CORES, 4, HID], FP8, tag="wdt")  # [o, c, ko, h]
                for c in range(N_CORES):
                    nc.sync.dma_start(
                        out=wdt4[:, c],
                        in_=ag_wd[c, :].rearrange(
                            "(ko o h) -> o ko h", ko=4, o=P, h=HID))
                wdt = wdt4[:].rearrange("o c ko h -> o (c ko) h")

                # ---------------- cached transposed gate/up weights -------------
                wg_sb = bigp.tile([P, KI, INNER], FP8, tag="wgc")
                wu_sb = bigp.tile([P, KI, INNER], FP8, tag="wuc")
                for c in range(N_CORES):
                    csl = slice(c * OSH, (c + 1) * OSH)
                    nc.sync.dma_start(out=wg_sb[:, :, csl], in_=ag_wg[c])
                    nc.sync.dma_start(out=wu_sb[:, :, csl], in_=ag_wu[c])

                # ---------------- main loop over token chunks -------------------
                for ch in range(main_chunks):
                    tsl = slice(ch * TC, (ch + 1) * TC)
                    cgb = ewp.tile([P, TC], F32, tag="cgb")
                    nc.gpsimd.partition_broadcast(cgb[:], cg_row[0:1, tsl])

                    prod = bigp.tile([P, KOG, TC], F32, tag="prod")
                    for m in range(KOG):
                        osl = slice(m * P, (m + 1) * P)
                        psg = pgp.tile([P, TC], F32, tag="pg")
                        for ki in range(KI):
                            nc.tensor.matmul(
                                psg[:], lhsT=wg_sb[:, ki, osl], rhs=xqT[:, ki, tsl],
                                start=(ki == 0), stop=(ki == KI - 1))
                        psu = pgp.tile([P, TC], F32, tag="pg")
                        for ki in range(KI):
                            nc.tensor.matmul(
                                psu[:], lhsT=wu_sb[:, ki, osl], rhs=xqT[:, ki, tsl],
                                start=(ki == 0), stop=(ki == KI - 1))
                        # gate*c_g -> silu -> * up_int
                        gsc = ewp.tile([P, TC], F32, tag="gsc")
                        nc.vector.tensor_tensor(
                            out=gsc[:], in0=psg[:], in1=cgb[:], op=A.mult)
                        gsil = ewp.tile([P, TC], F32, tag="gsil")
                        nc.scalar.activation(gsil[:], gsc[:], AF.Silu)
                        nc.vector.tensor_tensor(
                            out=prod[:, m], in0=gsil[:], in1=psu[:], op=A.mult)

                    # second act-quant: absmax over o (free sub-dim + partitions)
                    # 4-way split over kog so most reduces overlap the m-loop
                    nparts = 4
                    kq = KOG // nparts
                    maxr = ewp.tile([P, TC], F32, tag="maxr", bufs=1)
                    mr2 = ewp.tile([P, TC], F32, tag="maxr2", bufs=1)
                    for q in range(nparts):
                        dst = maxr if q == 0 else mr2
                        nc.vector.tensor_reduce(
                            out=dst[:],
                            in_=prod[:, q * kq:(q + 1) * kq].rearrange(
                                "p k t -> p t k"),
                            axis=mybir.AxisListType.X, op=A.max,
                            apply_absolute_value=True)
                        if q > 0:
                            nc.vector.tensor_tensor(
                                out=maxr[:], in0=maxr[:], in1=mr2[:], op=A.max)
                    maxg = ewp.tile([P, TC], F32, tag="maxg")
                    nc.gpsimd.partition_all_reduce(
                        maxg[:], maxr[:], channels=P,
                        reduce_op=bass_isa.ReduceOp.max)
                    nc.vector.tensor_scalar_max(maxg[:], maxg[:], 1e-5)
                    s2b = ewp.tile([P, TC], F32, tag="s2b")
                    nc.vector.reciprocal(out=s2b[:], in_=maxg[:])
                    nc.vector.tensor_scalar_mul(s2b[:], s2b[:], 127.0)

                    # quantize per k-slice so down matmuls consume them streaming
                    prodq = bigp.tile([P, KOG, TC], BF16, tag="prodq")
                    for k in range(0, KOG, 2):
                        nc.vector.tensor_tensor(
                            out=prod[:, k:k + 2], in0=prod[:, k:k + 2],
                            in1=s2b[:, None, :].to_broadcast((P, 2, TC)),
                            op=A.mult)
                        nc.vector.tensor_scalar(
                            out=prodq[:, k:k + 2], in0=prod[:, k:k + 2],
                            scalar1=MROUND, scalar2=-MROUND,
                            op0=A.add, op1=A.add)

                    # F_t column form for this chunk
                    fcol = tinyp.tile([P, MT], F32, tag="fcol")
                    for mt in range(MT):
                        nc.sync.dma_start(
                            out=fcol[:, mt:mt + 1],
                            in_=maxg[0:1, mt * P:(mt + 1) * P])
                    nc.vector.tensor_tensor(
                        out=fcol[:], in0=fcol[:],
                        in1=absm_c[:, ch * MT:(ch + 1) * MT], op=A.mult)
                    nc.vector.tensor_scalar_mul(fcol[:], fcol[:], bc_ud[:, 0:1])

                    # down projection
                    for mt in range(MT):
                        t0 = mt * P
                        for hh in range(2):
                            hsl = slice(hh * 512, (hh + 1) * 512)
                            psd = pdp.tile([P, 512], F32, tag="pd")
                            for kog in range(KOG):
                                nc.tensor.matmul(
                                    psd[:], lhsT=prodq[:, kog, t0:t0 + P],
                                    rhs=wdt[:, kog, hsl],
                                    start=(kog == 0), stop=(kog == KOG - 1))
                            osb = outpp.tile([P, 512], F32, tag="osb")
                            nc.scalar.activation(
                                osb[:], psd[:], AF.Copy, scale=fcol[:, mt:mt + 1])
                            nc.sync.dma_start(
                                out=out_r[ch * MT + mt][:, hsl], in_=osb[:])


            for _rep in range(reps):
                emit_body()

    nc.compile()
    return nc


_NC_CACHE = {}


def _get_nc():
    if "nc" not in _NC_CACHE:
        _NC_CACHE["nc"] = build_bass(sim_mode=False)
    return _NC_CACHE["nc"]


def make_in_maps(x, w_gate, w_up, w_down):
    x2 = np.ascontiguousarray(
        np.asarray(x, dtype=np.float32).reshape(N_CORES * T_CORE, HID))
    wg = np.asarray(w_gate, dtype=np.float32)
    wu = np.asarray(w_up, dtype=np.float32)
    wd = np.asarray(w_down, dtype=np.float32)
    in_maps = []
    for c in range(N_CORES):
        in_maps.append({
            "x_shard": np.ascontiguousarray(x2[c * T_CORE:(c + 1) * T_CORE]),
            "wg_shard": np.ascontiguousarray(wg[c * OSH:(c + 1) * OSH]),
            "wu_shard": np.ascontiguousarray(wu[c * OSH:(c + 1) * OSH]),
            "wd_shard": np.ascontiguousarray(wd[:, c * OSH:(c + 1) * OSH]),
        })
    return in_maps


def assemble_output(results):
    parts = [results[c]["out_shard"] for c in range(N_CORES)]
    return np.concatenate(parts, axis=0).reshape(4, 2048, HID)


def kernel(x, w_gate, w_up, w_down):
    from concourse.bass_utils import run_bass_kernel_spmd
    nc = _get_nc()
    in_maps = make_in_maps(x, w_gate, w_up, w_down)
    res = run_bass_kernel_spmd(nc, in_maps, list(range(N_CORES)), trace=False)
    return assemble_output(res.results)



# revision 22
# speedup vs baseline: 61.9101x; 61.9101x over previous
"""BitLinear FFN (BitNet b1.58) Trainium2 kernel, 8-core SPMD — v2.

Strategy: data-parallel over tokens (1024 tokens/core). Weight quantization
(+ transpose to contraction-major, + cast to fp8e4) is sharded 1/8 per core
and shared via pipelined AllGathers (gate+up in two column-halves so compute
can start after the first half lands; down weights in a third, needed last).

Numerics vs reference (target rel-err < 2e-2, achieved ~7e-3):
 - weight quantization exact: {-1,0,1} in fp8e4, global absmean scale via a
   tiny AllReduce of per-shard |w| sums.
 - first act-quant: exact int8 rounding, but the per-token dequant scale
   absmax/127 is folded into the stored bf16 activations. This makes every
   downstream dequant a per-tensor scalar: silu runs directly on PSUM via
   the Scalar engine (out = silu(psg * mean|w_g|)), and the final output is
   psd * (mean|w_u| * mean|w_d|).
 - second act-quant (int8 round of gate*up) is skipped: prod flows to the
   down matmul in bf16. Deviation = the reference's own quant noise (~0.7%
   rms) + bf16 rounding (~0.25%).
"""

import numpy as np

import concourse.bacc as bacc
import concourse.bass as bass
import concourse.bass_isa as bass_isa
import concourse.mybir as mybir
import concourse.tile as tile
from concourse.masks import make_identity

P = 128
HID = 1024
INNER = 4096
N_CORES = 8
T_CORE = 1024          # tokens per core
TC = 512               # token chunk
NCH = T_CORE // TC     # 2 chunks
KI = HID // P          # 8 contraction tiles for gate/up
KOG = INNER // P       # 32 contraction tiles for down
OSH = INNER // N_CORES  # 512 o-columns per core shard
MSH = OSH // P         # 4 m-tiles per shard
HSH = HID // P         # 8 row tiles in wd shard [1024, 512]
OHALF = OSH // 2       # 256: o-columns per gu AllGather piece
GUH = 2 * KI * P * OHALF  # fp8 bytes per core per gu piece (wg+wu): 524288

MROUND = 12582912.0    # 1.5 * 2**23: (v + M) - M == round-half-even(v)
W_ELEMS = float(INNER * HID)

F32 = mybir.dt.float32
BF16 = mybir.dt.bfloat16
F16 = mybir.dt.float16
FP8 = mybir.dt.float8e4

A = mybir.AluOpType
AF = mybir.ActivationFunctionType


def build_bass(sim_mode: bool = False, reps: int = 1):
    nc = bacc.Bacc(
        "TRN2", target_bir_lowering=False, debug=False,
        num_devices=N_CORES,
    )
    groups = [list(range(N_CORES))]

    x_d = nc.dram_tensor("x_shard", [T_CORE, HID], F32, kind="ExternalInput")
    wg_d = nc.dram_tensor("wg_shard", [OSH, HID], F32, kind="ExternalInput")
    wu_d = nc.dram_tensor("wu_shard", [OSH, HID], F32, kind="ExternalInput")
    wd_d = nc.dram_tensor("wd_shard", [HID, OSH], F32, kind="ExternalInput")
    out_d = nc.dram_tensor("out_shard", [T_CORE, HID], F32, kind="ExternalOutput")

    wg_r = wg_d.ap().rearrange("(po p) i -> po p i", p=P)    # [4, 128, 1024]
    wu_r = wu_d.ap().rearrange("(po p) i -> po p i", p=P)
    wd_r = wd_d.ap().rearrange("(hs p) o -> hs p o", p=P)    # [8, 128, 512]
    x_r = x_d.ap().rearrange("(n p) i -> n p i", p=P)        # [8, 128, 1024]
    out_r = out_d.ap().rearrange("(n p) h -> n p h", p=P)

    with tile.TileContext(nc) as tc:
        with (
            tc.tile_pool(name="const", bufs=1) as constp,
            tc.tile_pool(name="stream", bufs=2) as streamp,
            tc.tile_pool(name="qp", bufs=2) as qp,
            tc.tile_pool(name="big", bufs=1) as bigp,
            tc.tile_pool(name="prodp", bufs=2) as prodp,
            tc.tile_pool(name="stg", bufs=1) as stgp,
            tc.tile_pool(name="wdtp", bufs=2) as wdtp,
            tc.tile_pool(name="ew", bufs=2) as ewp,
            tc.tile_pool(name="mx", bufs=2) as mxp,
            tc.tile_pool(name="outp", bufs=2) as outpp,
            tc.tile_pool(name="tiny", bufs=2) as tinyp,
            tc.tile_pool(name="pg", bufs=2, space="PSUM") as pgp,
            tc.tile_pool(name="pd", bufs=4, space="PSUM") as pdp,
            tc.tile_pool(name="pt", bufs=2, space="PSUM") as ptp,
            tc.tile_pool(name="dram", bufs=1, space="DRAM") as dramp,
        ):
            ident = constp.tile([P, P], F16)
            make_identity(nc, ident)
            ones_col = constp.tile([P, 1], F32)
            nc.gpsimd.memset(ones_col[:], 1.0)

            def emit_body():
                # ---- |w| partial sums: ACT abs (in-place) + row-sum accum ----
                parts = constp.tile([P, 16], F32)
                srcs = [(wg_r, 4, HID), (wu_r, 4, HID), (wd_r, HSH, OSH)]
                idx = 0
                for src, n_sub, width in srcs:
                    for po in range(n_sub):
                        wld = streamp.tile([P, HID], F32, tag="wld")
                        nc.sync.dma_start(out=wld[:, :width], in_=src[po])
                        nc.scalar.activation(
                            wld[:, :width], wld[:, :width], AF.Abs,
                            accum_out=parts[:, idx:idx + 1])
                        idx += 1

                psums16 = ptp.tile([1, 16], F32, tag="pt")
                nc.tensor.matmul(psums16[:], lhsT=ones_col[:], rhs=parts[:],
                                 start=True, stop=True)
                sums16 = tinyp.tile([1, 16], F32, tag="s16")
                nc.scalar.activation(sums16[:], psums16[:], AF.Copy)

                # tiny AllReduce of the 16 |w| partial sums (64 B); the
                # group-combine happens after, off the critical path.
                sums_in = dramp.tile([1, 16], F32)
                sums_out = dramp.tile([1, 16], F32, addr_space="Shared")
                nc.gpsimd.dma_start(out=sums_in[:], in_=sums16[:])
                if sim_mode:
                    nc.gpsimd.dma_start(out=sums_out[:], in_=sums_in[:])
                else:
                    nc.gpsimd.collective_compute(
                        "AllReduce", A.add, replica_groups=groups,
                        ins=[sums_in[:]], outs=[sums_out[:]])
                sums_all = tinyp.tile([1, 16], F32, tag="sall")
                nc.gpsimd.dma_start(out=sums_all[:], in_=sums_out[:])

                # ---- x: exact int8 quant with absmax/127 folded in (fp16) ----
                xqTs = bigp.tile([P, KI, T_CORE], F16, tag="xqT")  # [i, t]
                for ts in range(KI):
                    x_sb = streamp.tile([P, HID], F32, tag="wld")
                    nc.sync.dma_start(out=x_sb[:], in_=x_r[ts])
                    am = tinyp.tile([P, 1], F32, tag="am")
                    nc.vector.tensor_reduce(
                        out=am[:], in_=x_sb[:], axis=mybir.AxisListType.X,
                        op=A.max, apply_absolute_value=True)
                    nc.vector.tensor_scalar_max(am[:], am[:], 1e-5)
                    s1c = tinyp.tile([P, 1], F32, tag="s1c")
                    nc.vector.reciprocal(out=s1c[:], in_=am[:])
                    nc.vector.tensor_scalar_mul(s1c[:], s1c[:], 127.0)
                    am127 = tinyp.tile([P, 1], F32, tag="am127")
                    nc.vector.tensor_scalar_mul(am127[:], am[:], 1.0 / 127.0)
                    nc.vector.tensor_scalar(
                        out=x_sb[:], in0=x_sb[:], scalar1=s1c[:, 0:1],
                        scalar2=MROUND, op0=A.mult, op1=A.add)
                    xq_b = qp.tile([P, HID], F16, tag="xqb")
                    nc.vector.tensor_scalar(
                        out=xq_b[:], in0=x_sb[:], scalar1=-MROUND,
                        scalar2=am127[:, 0:1], op0=A.add, op1=A.mult)
                    for ki in range(KI):
                        pt_t = ptp.tile([P, P], F16, tag="pt")
                        nc.tensor.transpose(
                            pt_t[:], xq_b[:, ki * P:(ki + 1) * P], ident[:])
                        nc.scalar.activation(
                            xqTs[:, ki, ts * P:(ts + 1) * P], pt_t[:], AF.Copy)

                # ---- per-tensor scales from the AllReduced sums ----
                sums3 = tinyp.tile([1, 3], F32, tag="s3")
                for j, sl in enumerate((slice(0, 4), slice(4, 8), slice(8, 16))):
                    nc.vector.tensor_reduce(
                        out=sums3[:, j:j + 1], in_=sums_all[:, sl],
                        axis=mybir.AxisListType.X, op=A.add)
                mcl = tinyp.tile([1, 3], F32, tag="mcl")
                nc.vector.tensor_scalar(
                    out=mcl[:], in0=sums3[:], scalar1=1.0 / W_ELEMS,
                    scalar2=1e-5, op0=A.mult, op1=A.max)
                mclb = constp.tile([P, 3], F32)
                nc.gpsimd.partition_broadcast(mclb[:], mcl[0:1, :])
                swb = constp.tile([P, 3], F32)     # 1/mcl = quant scales
                nc.vector.reciprocal(out=swb[:], in_=mclb[:])
                fcol = constp.tile([P, 1], F32)    # mcl_u * mcl_d / 127
                nc.vector.tensor_tensor(
                    out=fcol[:], in0=mclb[:, 1:2], in1=mclb[:, 2:3], op=A.mult)
                nc.vector.tensor_scalar_mul(fcol[:], fcol[:], 1.0 / 127.0)

                # ---- quantize + transpose weight shards, stage, AllGather ----
                shared_as = "Local" if sim_mode else "Shared"
                stg_gu = dramp.tile([2, GUH], FP8)
                stg_wd = dramp.tile([OSH * HID], FP8)
                agt_gu = [
                    dramp.tile([N_CORES, GUH], FP8, addr_space=shared_as,
                               name=f"agt_gu{h}")
                    for h in range(2)
                ]
                ag_wd = dramp.tile([N_CORES, OSH * HID], FP8,
                                   addr_space=shared_as)

                def do_cc(in_ap, out_ap):
                    if sim_mode:
                        n = out_ap.shape[0]
                        for c in range(n):
                            nc.gpsimd.dma_start(out=out_ap[c], in_=in_ap)
                    else:
                        nc.gpsimd.collective_compute(
                            "AllGather", A.bypass, replica_groups=groups,
                            ins=[in_ap], outs=[out_ap[:]])

                def quant_tile(src_sb, w_idx, width):
                    """bf16 <- clip(round(src * (1/mcl)), -1, 1); src clobbered."""
                    nc.vector.tensor_scalar(
                        out=src_sb, in0=src_sb, scalar1=swb[:, w_idx:w_idx + 1],
                        scalar2=MROUND, op0=A.mult, op1=A.add)
                    wq_b = qp.tile([P, HID], F16, tag="qb")
                    nc.vector.tensor_scalar(
                        out=wq_b[:, :width], in0=src_sb, scalar1=-MROUND,
                        scalar2=1.0, op0=A.add, op1=A.min)
                    nc.vector.tensor_scalar_max(
                        wq_b[:, :width], wq_b[:, :width], -1.0)
                    return wq_b

                # gate/up: two column-halves, each staged+gathered separately.
                # stage layout per half (DRAM): (w, ki, i, o256)
                stage_gu = stgp.tile([P, 2, 2, KI, OHALF], FP8, tag="sgu")
                for h in range(2):
                    for w_idx, src in ((0, wg_r), (1, wu_r)):
                        for po2 in range(2):
                            po = 2 * h + po2
                            wld = streamp.tile([P, HID], F32, tag="wld")
                            nc.sync.dma_start(out=wld[:], in_=src[po])
                            wq_b = quant_tile(wld[:], w_idx, HID)
                            for ki in range(KI):
                                pt_t = ptp.tile([P, P], F16, tag="pt")
                                nc.tensor.transpose(
                                    pt_t[:], wq_b[:, ki * P:(ki + 1) * P],
                                    ident[:])
                                nc.scalar.activation(
                                    stage_gu[:, h, w_idx, ki,
                                             po2 * P:(po2 + 1) * P],
                                    pt_t[:], AF.Copy)
                    nc.sync.dma_start(
                        out=stg_gu[h].rearrange(
                            "(w ki i o) -> i w ki o", w=2, ki=KI, i=P, o=OHALF),
                        in_=stage_gu[:, h])
                    do_cc(stg_gu[h], agt_gu[h])

                # down: stage layout (ko, o, h) as [4, 128, 1024]
                stage_wd = stgp.tile([P, 4, HID], FP8, tag="swd")
                for hs in range(HSH):
                    wld = streamp.tile([P, HID], F32, tag="wld")
                    nc.sync.dma_start(out=wld[:, :OSH], in_=wd_r[hs])
                    wq_b = quant_tile(wld[:, :OSH], 2, OSH)
                    for ko in range(4):
                        pt_t = ptp.tile([P, P], F16, tag="pt")
                        nc.tensor.transpose(
                            pt_t[:], wq_b[:, ko * P:(ko + 1) * P], ident[:])
                        nc.scalar.activation(
                            stage_wd[:, ko, hs * P:(hs + 1) * P],
                            pt_t[:], AF.Copy)
                nc.sync.dma_start(
                    out=stg_wd[:].rearrange(
                        "(ko o h) -> o ko h", ko=4, o=P, h=HID),
                    in_=stage_wd[:])
                do_cc(stg_wd[:], ag_wd)

                # ---- read gathered gate/up weights back into SBUF ----
                wg_sb = bigp.tile([P, KI, INNER], FP8, tag="wgc")
                wu_sb = bigp.tile([P, KI, INNER], FP8, tag="wuc")
                for h in range(2):
                    for c in range(N_CORES):
                        src = agt_gu[h][c].rearrange(
                            "(w ki i o) -> w i ki o", w=2, ki=KI, i=P, o=OHALF)
                        base = c * OSH + h * OHALF
                        nc.sync.dma_start(
                            out=wg_sb[:, :, base:base + OHALF], in_=src[0])
                        nc.sync.dma_start(
                            out=wu_sb[:, :, base:base + OHALF], in_=src[1])

                # ---- gate/up sweep: [o, t] layout, half-major for overlap ----
                # prod slot layout: half hh's 16 m-tiles land contiguously at
                # slots [16*hh, 16*hh+16) so the absmax reduce reads a
                # contiguous kog range. slot(s, m) = 16*(m//2) + 2*s + m%2.
                prod_t = [
                    prodp.tile([P, KOG, TC], F16, tag="prod",
                               name=f"prod{ch}")
                    for ch in range(NCH)
                ]
                mx_t = [
                    mxp.tile([P, TC], F32, tag="mx", name=f"mx{ch}")
                    for ch in range(NCH)
                ]
                s2_t = [
                    mxp.tile([P, TC], F32, tag="s2", name=f"s2{ch}")
                    for ch in range(NCH)
                ]
                fc2_t = [
                    tinyp.tile([P, TC // P], F32, tag="fc2", name=f"fc2{ch}")
                    for ch in range(NCH)
                ]

                def reduce_absmax(ch, hh, dst):
                    nc.vector.tensor_reduce(
                        out=dst,
                        in_=prod_t[ch][:, 16 * hh:16 * hh + 16, :].rearrange(
                            "p k t -> p t k"),
                        axis=mybir.AxisListType.X, op=A.max,
                        apply_absolute_value=True)

                for hh in range(2):
                    for ch in range(NCH):
                        tsl = slice(ch * TC, (ch + 1) * TC)
                        for s in range(N_CORES):
                            for m in (2 * hh, 2 * hh + 1):
                                col = s * OSH + m * P
                                kslot = 16 * hh + 2 * s + (m % 2)
                                psg = pgp.tile([P, TC], F32, tag="pg")
                                for ki in range(KI):
                                    nc.tensor.matmul(
                                        psg[:], lhsT=wg_sb[:, ki, col:col + P],
                                        rhs=xqTs[:, ki, tsl],
                                        start=(ki == 0), stop=(ki == KI - 1))
                                psu = pgp.tile([P, TC], F32, tag="pg")
                                for ki in range(KI):
                                    nc.tensor.matmul(
                                        psu[:], lhsT=wu_sb[:, ki, col:col + P],
                                        rhs=xqTs[:, ki, tsl],
                                        start=(ki == 0), stop=(ki == KI - 1))
                                # gsil = silu(psg * mcl_g) on ScalarE; DVE
                                # may read at most one PSUM operand, so gsil
                                # lands in SBUF.
                                gsil = ewp.tile([P, TC], F32, tag="gsil")
                                nc.scalar.activation(
                                    gsil[:], psg[:], AF.Silu,
                                    scale=mclb[:, 0:1])
                                nc.vector.tensor_tensor(
                                    out=prod_t[ch][:, kslot, :], in0=gsil[:],
                                    in1=psu[:], op=A.mult)
                        if hh == 0:
                            reduce_absmax(ch, 0, mx_t[ch][:])
                        else:
                            # second act-quant: absmax over INNER per token,
                            # then prodq = round(prod * 127 / absmax) in place
                            mx1 = ewp.tile([P, TC], F32, tag="mx1", bufs=1)
                            reduce_absmax(ch, 1, mx1[:])
                            mx = mx_t[ch]
                            nc.vector.tensor_tensor(
                                out=mx[:], in0=mx[:], in1=mx1[:], op=A.max)
                            s2b = s2_t[ch]
                            nc.gpsimd.partition_all_reduce(
                                s2b[:], mx[:], channels=P,
                                reduce_op=bass_isa.ReduceOp.max)
                            nc.vector.tensor_scalar_max(s2b[:], s2b[:], 1e-5)
                            # per-token output scale F = absmax*mclu*mcld/127
                            # (read the absmax rows before the in-place
                            # reciprocal below turns s2b into 1/absmax)
                            fc2 = fc2_t[ch]
                            for tt in range(TC // P):
                                nc.sync.dma_start(
                                    out=fc2[:, tt:tt + 1],
                                    in_=s2b[0:1, tt * P:(tt + 1) * P])
                            nc.vector.tensor_scalar(
                                out=fc2[:], in0=fc2[:], scalar1=fcol[:, 0:1],
                                scalar2=None, op0=A.mult)
                            nc.vector.reciprocal(out=s2b[:], in_=s2b[:])
                            for g in range(KOG // 4):
                                ksl = slice(4 * g, 4 * g + 4)
                                qtmp = ewp.tile([P, 4, TC], F32, tag="qtmp",
                                                bufs=1)
                                nc.vector.scalar_tensor_tensor(
                                    out=qtmp[:], in0=prod_t[ch][:, ksl, :],
                                    scalar=127.0,
                                    in1=s2b[:, None, :].to_broadcast(
                                        (P, 4, TC)),
                                    op0=A.mult, op1=A.mult)
                                nc.vector.tensor_scalar(
                                    out=prod_t[ch][:, ksl, :], in0=qtmp[:],
                                    scalar1=MROUND, scalar2=-MROUND,
                                    op0=A.add, op1=A.add)

                # ---- down projection: stream wd tiles, 4 psum banks ----
                for ch in range(NCH):
                    for hh in range(2):
                        hsl = slice(hh * 512, (hh + 1) * 512)
                        psds = [pdp.tile([P, 512], F32, tag="pd",
                                         name=f"psd{tt}")
                                for tt in range(MSH)]
                        for c in range(N_CORES):
                            wdt_c = wdtp.tile([P, 4, 512], FP8, tag="wdt")
                            nc.sync.dma_start(
                                out=wdt_c[:],
                                in_=ag_wd[c].rearrange(
                                    "(ko o h) -> o ko h",
                                    ko=4, o=P, h=HID)[:, :, hsl])
                            for tt in range(MSH):
                                t0 = tt * P
                                for ko in range(4):
                                    kslot = 16 * (ko // 2) + 2 * c + (ko % 2)
                                    nc.tensor.matmul(
                                        psds[tt][:],
                                        lhsT=prod_t[ch][:, kslot, t0:t0 + P],
                                        rhs=wdt_c[:, ko, :],
                                        start=(c == 0 and ko == 0),
                                        stop=(c == N_CORES - 1 and ko == 3))
                        for tt in range(MSH):
                            osb = outpp.tile([P, 512], F32, tag="osb")
                            nc.scalar.activation(
                                osb[:], psds[tt][:], AF.Copy,
                                scale=fc2_t[ch][:, tt:tt + 1])
                            nc.sync.dma_start(
                                out=out_r[ch * MSH + tt][:, hsl], in_=osb[:])

            for _rep in range(reps):
                emit_body()

    nc.compile()
    return nc


_NC_CACHE = {}


def _get_nc():
    if "nc" not in _NC_CACHE:
        _NC_CACHE["nc"] = build_bass(sim_mode=False)
    return _NC_CACHE["nc"]


def make_in_maps(x, w_gate, w_up, w_down):
    x2 = np.ascontiguousarray(
        np.asarray(x, dtype=np.float32).reshape(N_CORES * T_CORE, HID))
    wg = np.asarray(w_gate, dtype=np.float32)
    wu = np.asarray(w_up, dtype=np.float32)
    wd = np.asarray(w_down, dtype=np.float32)
    in_maps = []
    for c in range(N_CORES):
        in_maps.append({
            "x_shard": np.ascontiguousarray(x2[c * T_CORE:(c + 1) * T_CORE]),
            "wg_shard": np.ascontiguousarray(wg[c * OSH:(c + 1) * OSH]),
            "wu_shard": np.ascontiguousarray(wu[c * OSH:(c + 1) * OSH]),
            "wd_shard": np.ascontiguousarray(wd[:, c * OSH:(c + 1) * OSH]),
        })
    return in_maps


def assemble_output(results):
    parts = [results[c]["out_shard"] for c in range(N_CORES)]
    return np.concatenate(parts, axis=0).reshape(4, 2048, HID)


def kernel(x, w_gate, w_up, w_down):
    from concourse.bass_utils import run_bass_kernel_spmd
    nc = _get_nc()
    in_maps = make_in_maps(x, w_gate, w_up, w_down)
    res = run_bass_kernel_spmd(nc, in_maps, list(range(N_CORES)), trace=False)
    return assemble_output(res.results)
